# revision 23
# baseline (speedup 1.0000x reference)
"""AugmentedTripletLoss Trainium2 kernel — 8-core SPMD, row-sharded.

Math (matches reference):
  d2[i,j]   = sq_i + sq_j - 2*X@X.T
  ap_i      = sqrt(clip(max_{same class}(d2), 1e-12))
  an_i      = min( sqrt(clip(min_{diff class}(d2), 1e-12)),
                   clip(sqrt(clip(min_c(sq_i + csq_c - 2*x_i.cn_c), 0)), 1e-12) )
  loss      = mean(relu(1 + ap - an))

Device strategy (per core, 512 query rows):
  One bf16 matmul with an augmented contraction dim of 896 = 768 (X^T)
  + 2 (sq_j split hi/lo bf16) + 100 (BIG*onehot(class)) + 26 zero-pad
  produces u = -2*S + sq_j + BIG*[same class] directly in PSUM, so the
  masked max/min reductions are single fused DVE passes:
      ap2 = max_j u - BIG + sq_i,   an2 = min_j u + sq_i.
  The monotonicity of sqrt/clip lets all sqrt happen on [512]-vectors.
  Centers ride the same query lhsT with rhs = [cn^T; csq_hi; csq_lo; 0].
  Final: per-core sum -> AllReduce -> /N.
"""
import os
import sys

for _p in ("/opt/trn_rl_repo", "/root/.axon_site"):
    if _p not in sys.path:
        sys.path.insert(0, _p)

import numpy as np

import concourse.bass as bass
import concourse.bacc as bacc
import concourse.mybir as mybir
from concourse.tile import TileContext
from concourse.masks import make_identity
from concourse.bass_utils import run_bass_kernel_spmd

F32 = mybir.dt.float32
BF16 = mybir.dt.bfloat16
I32 = mybir.dt.int32
ALU = mybir.AluOpType
ACTF = mybir.ActivationFunctionType
AX = mybir.AxisListType

N_CORES = 8
N, D, P = 4096, 768, 100
NQ = N // N_CORES              # 512 query rows per core
NXT = N // 128                 # 32 x-tiles of 128 rows
MQ = NQ // 128                 # 4 query m-tiles
BIG = 16384.0
MARGIN = 1.0
KA = 7                         # augmented contraction tiles of 128 (896 total)
JGRP = 4                       # x-tiles per column group (512 cols)
NJ = NXT // JGRP               # 8 column groups

_nc_cache = None


def _build():
    stage_lim = int(os.environ.get("KSTAGE", "9"))
    nc = bacc.Bacc("TRN2", target_bir_lowering=False, num_devices=N_CORES)

    x_h = nc.declare_dram_parameter("x", [N, D], F32, isOutput=False)
    xq_h = nc.declare_dram_parameter("xq", [NQ, D], F32, isOutput=False)
    tgt_h = nc.declare_dram_parameter("tgt", [N], F32, isOutput=False)
    tq_h = nc.declare_dram_parameter("tq", [NQ], F32, isOutput=False)
    cen_h = nc.declare_dram_parameter("center", [P, D], F32, isOutput=False)
    loss_h = nc.declare_dram_parameter("loss", [1, 1], F32, isOutput=True)
    dbg_on = os.environ.get("KDBG", "0") == "1"
    dbg_h = nc.declare_dram_parameter("dbg", [128, 64], F32, isOutput=True) if dbg_on else None
    cc_in = nc.dram_tensor("cc_in", [1, 1], F32)
    cc_out = nc.dram_tensor("cc_out", [1, 1], F32, addr_space="Shared")

    with TileContext(nc) as tc:
        from contextlib import ExitStack

        with ExitStack() as ctx:
            const = ctx.enter_context(tc.tile_pool(name="const", bufs=1))
            keyp = ctx.enter_context(tc.tile_pool(name="key", bufs=1))
            stage = ctx.enter_context(tc.tile_pool(name="stage", bufs=8))
            small = ctx.enter_context(tc.tile_pool(name="small", bufs=2))
            pmain = ctx.enter_context(tc.tile_pool(name="pmain", bufs=5, space="PSUM"))
            ptrp = ctx.enter_context(tc.tile_pool(name="ptrp", bufs=2, space="PSUM"))
            psmall = ctx.enter_context(tc.tile_pool(name="psmall", bufs=1, space="PSUM"))

            # ---------- constants ----------
            ident = const.tile([128, 128], BF16)
            make_identity(nc, ident[:])
            iota_i = const.tile([128, 1], I32)
            nc.gpsimd.iota(iota_i[:], pattern=[[1, 1]], base=0, channel_multiplier=1)
            iota_a = const.tile([128, 1], F32)    # class ids for partitions 0..95
            nc.vector.tensor_copy(iota_a[:], iota_i[:])
            iota_i2 = const.tile([128, 1], I32)
            nc.gpsimd.iota(iota_i2[:], pattern=[[1, 1]], base=-2, channel_multiplier=1)
            iota_b = const.tile([128, 1], F32)    # class ids for partitions 98..101
            nc.vector.tensor_copy(iota_b[:], iota_i2[:])
            nc.vector.memset(iota_b[96:98, :], -1.0)
            zeros_bf = const.tile([128, 512], BF16)
            nc.vector.memset(zeros_bf[:], 0.0)
            eps30 = const.tile([128, 1], F32)
            nc.vector.memset(eps30[:], 1e-30)
            marg = const.tile([128, 1], F32)
            nc.vector.memset(marg[:], MARGIN)

            # ---------- key-side tiles ----------
            kT = [keyp.tile([128, N], BF16, tag=f"kT{d}", name=f"kT{d}") for d in range(KA)]

            tgt_b = keyp.tile([128, N], F32, tag="tgtb")
            nc.gpsimd.dma_start(
                out=tgt_b[:], in_=bass.AP(tensor=tgt_h, offset=0, ap=[[0, 128], [1, N]])
            )
            nc.vector.tensor_scalar(
                out=kT[6][0:96, :], in0=tgt_b[0:96, :],
                scalar1=iota_a[0:96, 0:1], scalar2=BIG,
                op0=ALU.is_equal, op1=ALU.mult,
            )
            nc.vector.tensor_scalar(
                out=kT[6][96:128, :], in0=tgt_b[96:128, :],
                scalar1=iota_b[96:128, 0:1], scalar2=BIG,
                op0=ALU.is_equal, op1=ALU.mult,
            )

            # ---------- query-side tiles ----------
            qT = [const.tile([128, NQ], BF16, tag=f"qT{d}", name=f"qT{d}") for d in range(KA)]
            tq_b = const.tile([128, NQ], F32)
            nc.gpsimd.dma_start(
                out=tq_b[:], in_=bass.AP(tensor=tq_h, offset=0, ap=[[0, 128], [1, NQ]])
            )
            nc.vector.tensor_scalar(
                out=qT[6][0:96, :], in0=tq_b[0:96, :],
                scalar1=iota_a[0:96, 0:1], scalar2=None, op0=ALU.is_equal,
            )
            nc.vector.tensor_scalar(
                out=qT[6][96:128, :], in0=tq_b[96:128, :],
                scalar1=iota_b[96:128, 0:1], scalar2=None, op0=ALU.is_equal,
            )
            nc.vector.memset(qT[6][96:98, :], 1.0)

            sq_q = const.tile([128, MQ], F32)       # query row norms
            sq_dump = stage.tile([128, D], BF16, tag="sqdump")
            for m in range(MQ):
                qxb = stage.tile([128, D], BF16, tag="qxb")
                nc.gpsimd.dma_start(out=qxb[:], in_=xq_h[m * 128 : (m + 1) * 128, :])
                for d in range(6):
                    nc.sync.dma_start(
                        out=qT[d][:, m * 128 : (m + 1) * 128],
                        in_=qxb[:, d * 128 : (d + 1) * 128],
                        transpose=True,
                    )
                nc.scalar.activation(
                    out=sq_dump[:], in_=qxb[:], func=ACTF.Square,
                    accum_out=sq_q[:, m : m + 1],
                )
            for d in range(6):
                nc.vector.tensor_scalar_mul(qT[d][:], qT[d][:], -2.0)

            # ---------- centers ----------
            ct32 = small.tile([128, D], F32, tag="ct32")
            nc.vector.memset(ct32[:], 0.0)
            nc.gpsimd.dma_start(out=ct32[0:P, :], in_=cen_h[:, :])
            eps30 = const.tile([128, 1], F32)
            nc.vector.memset(eps30[:], 1e-30)
            marg = const.tile([128, 1], F32)
            nc.vector.memset(marg[:], MARGIN)
            csum = const.tile([128, 1], F32)
            cdump = small.tile([128, D], F32, tag="cdump")
            nc.scalar.activation(
                out=cdump[:], in_=ct32[:], func=ACTF.Square, accum_out=csum[:]
            )
            cnorm = const.tile([128, 1], F32)
            nc.scalar.activation(out=cnorm[:], in_=csum[:], func=ACTF.Sqrt, bias=eps30[:])
            rnorm = const.tile([128, 1], F32)
            nc.vector.reciprocal(rnorm[:], cnorm[:])
            cn32 = small.tile([128, D], F32, tag="cn32")
            nc.vector.tensor_scalar(
                out=cn32[:], in0=ct32[:], scalar1=rnorm[:, 0:1], scalar2=None,
                op0=ALU.mult,
            )
            csq = const.tile([128, 1], F32)
            nc.scalar.activation(
                out=cdump[:], in_=cn32[:], func=ACTF.Square, accum_out=csq[:]
            )
            cnb = small.tile([128, D], BF16, tag="cnb")
            nc.vector.tensor_copy(cnb[:], cn32[:])

            cT = [const.tile([128, P], BF16, tag=f"cT{d}", name=f"cT{d}") for d in range(KA)]
            nc.vector.memset(cT[6][:], 0.0)
            for d in range(6):
                pt = psmall.tile([128, 128], BF16, tag="ps")
                nc.tensor.transpose(pt[:], cnb[:, d * 128 : (d + 1) * 128], ident[:])
                nc.vector.tensor_copy(cT[d][:], pt[:, 0:P])
            # csq hi/lo row block
            chl = const.tile([128, 128], BF16)
            nc.vector.memset(chl[:], 0.0)
            nc.vector.tensor_copy(chl[:, 0:1], csq[:])
            chl32 = const.tile([128, 1], F32)
            nc.vector.tensor_copy(chl32[:], chl[:, 0:1])
            nc.vector.tensor_sub(chl[:, 1:2], csq[:], chl32[:])
            ptc = psmall.tile([128, 128], BF16, tag="ps")
            nc.tensor.transpose(ptc[:], chl[:], ident[:])
            nc.vector.tensor_copy(cT[6][96:98, :], ptc[0:2, 0:P])

            # center GEMM: w = -2*x.cn + csq  -> running min into wmin
            wmin = const.tile([128, MQ], F32)
            nc.vector.memset(wmin[:], 3.0e38)
            scrC = small.tile([128, P], BF16, tag="scrC")
            for m in range(MQ):
                pc = psmall.tile([128, P], F32, tag="ps")
                for d in range(KA):
                    nc.tensor.matmul(
                        pc[:], qT[d][:, m * 128 : (m + 1) * 128], cT[d][:, 0:P],
                        start=(d == 0), stop=(d == KA - 1),
                    )
                if use_ttr:
                    nc.vector.tensor_tensor_reduce(
                        out=scrC[:], in0=pc[:], in1=zeros_bf[:, 0:P], scale=1.0,
                        scalar=wmin[:, m : m + 1], op0=ALU.add, op1=ALU.min,
                        accum_out=wmin[:, m : m + 1],
                    )
                else:
                    tmpm = small.tile([128, 1], F32, tag="tmpm")
                    nc.vector.tensor_reduce(out=tmpm[:], in_=pc[:], axis=AX.X, op=ALU.min)
                    nc.vector.tensor_tensor(out=wmin[:, m : m + 1], in0=wmin[:, m : m + 1], in1=tmpm[:], op=ALU.min)

            # ---------- main stream: load X, transpose, sq, GEMM, reduce ----------
            apmax = const.tile([128, MQ], F32)
            anmin = const.tile([128, MQ], F32)
            apcols = [const.tile([128, NJ], F32, name=f"apcols{m}") for m in range(MQ)]
            ancols = [const.tile([128, NJ], F32, name=f"ancols{m}") for m in range(MQ)]
            nc.vector.memset(apmax[:], -3.0e38)
            nc.vector.memset(anmin[:], 3.0e38)
            for m in range(MQ):
                nc.vector.memset(apcols[m][:], -3.0e38)
                nc.vector.memset(ancols[m][:], 3.0e38)
            sq_cols = const.tile([128, NXT], F32)
            scr = small.tile([128, 512], BF16, tag="scr")

            for J in range(NJ if stage_lim >= 2 else 0):
                xbs = []
                for jj in range(JGRP):
                    j = J * JGRP + jj
                    xb = stage.tile([128, D], BF16, tag="xb", name=f"xb{j}")
                    nc.gpsimd.dma_start(out=xb[:], in_=x_h[j * 128 : (j + 1) * 128, :])
                    nc.scalar.activation(
                        out=sq_dump[:], in_=xb[:], func=ACTF.Square,
                        accum_out=sq_cols[:, j : j + 1],
                    )
                    xbs.append(xb)
                for d in range(6):
                    ptt = ptrp.tile([128, 512], BF16, tag="ptt", name=f"ptt{J}_{d}")
                    for jj in range(JGRP):
                        nc.tensor.transpose(
                            ptt[:, jj * 128 : (jj + 1) * 128],
                            xbs[jj][:, d * 128 : (d + 1) * 128],
                            ident[:],
                        )
                    ceng = nc.vector if d % 2 == 0 else nc.scalar
                    if d % 2 == 0:
                        nc.vector.tensor_copy(
                            out=kT[d][:, J * 512 : (J + 1) * 512], in_=ptt[:]
                        )
                    else:
                        nc.scalar.copy(
                            out=kT[d][:, J * 512 : (J + 1) * 512], in_=ptt[:]
                        )
                # sq -> bf16 hi/lo, interleaved (hi0,lo0,hi1,lo1,...) for transpose
                # hi_j at col 32j, lo_j at col 32j+1 -> transposed rows land at
                # partition bases {0,32,64,96}, all 32-aligned for the copies.
                hilo = stage.tile([128, 128], BF16, tag="hilo")
                nc.vector.memset(hilo[:], 0.0)
                hvv = hilo[:].rearrange("p (g t) -> p g t", t=32)
                sq4 = sq_cols[:, J * JGRP : (J + 1) * JGRP]
                sq4v = sq4.rearrange("p (j o) -> p j o", o=1)
                nc.vector.tensor_copy(hvv[:, :, 0:1], sq4v)
                hi32 = stage.tile([128, JGRP], F32, tag="hi32")
                nc.vector.tensor_copy(hi32[:], hvv[:, :, 0:1].rearrange("p j o -> p (j o)"))
                nc.vector.tensor_sub(
                    hvv[:, :, 1:2], sq4v, hi32[:].rearrange("p (j o) -> p j o", o=1)
                )
                pst = psmall.tile([128, 128], BF16, tag="ps")
                nc.tensor.transpose(pst[:], hilo[:], ident[:])
                for jj in range(JGRP):
                    j = J * JGRP + jj
                    nc.vector.tensor_copy(
                        out=kT[6][96:98, j * 128 : (j + 1) * 128],
                        in_=pst[32 * jj : 32 * jj + 2, :],
                    )

                for m in range(MQ):
                    pt = pmain.tile([128, 512], F32, tag="mm")
                    for d in range(KA):
                        nc.tensor.matmul(
                            pt[:],
                            qT[d][:, m * 128 : (m + 1) * 128],
                            kT[d][:, J * 512 : (J + 1) * 512],
                            start=(d == 0), stop=(d == KA - 1),
                        )
                    nc.vector.tensor_reduce(
                        out=apcols[m][:, J : J + 1], in_=pt[:], axis=AX.X, op=ALU.max
                    )
                    nc.vector.tensor_reduce(
                        out=ancols[m][:, J : J + 1], in_=pt[:], axis=AX.X, op=ALU.min
                    )

            # ---------- finals ----------
            for m in range(MQ):
                nc.vector.tensor_reduce(
                    out=apmax[:, m : m + 1], in_=apcols[m][:], axis=AX.X, op=ALU.max
                )
                nc.vector.tensor_reduce(
                    out=anmin[:, m : m + 1], in_=ancols[m][:], axis=AX.X, op=ALU.min
                )
            ap2 = const.tile([128, MQ], F32)
            nc.vector.tensor_scalar_add(ap2[:], apmax[:], -BIG)
            nc.vector.tensor_add(ap2[:], ap2[:], sq_q[:])
            nc.vector.tensor_scalar_max(ap2[:], ap2[:], 1e-12)
            ap_d = const.tile([128, MQ], F32)
            nc.scalar.activation(out=ap_d[:], in_=ap2[:], func=ACTF.Sqrt)

            an2 = const.tile([128, MQ], F32)
            nc.vector.tensor_add(an2[:], anmin[:], sq_q[:])
            nc.vector.tensor_scalar_max(an2[:], an2[:], 1e-12)
            an_d = const.tile([128, MQ], F32)
            nc.scalar.activation(out=an_d[:], in_=an2[:], func=ACTF.Sqrt)

            dc2 = const.tile([128, MQ], F32)
            nc.vector.tensor_add(dc2[:], wmin[:], sq_q[:])
            nc.vector.tensor_scalar_max(dc2[:], dc2[:], 0.0)
            dc_d = const.tile([128, MQ], F32)
            nc.scalar.activation(out=dc_d[:], in_=dc2[:], func=ACTF.Sqrt)
            nc.vector.tensor_scalar_max(dc_d[:], dc_d[:], 1e-12)

            an_f = const.tile([128, MQ], F32)
            nc.vector.tensor_tensor(out=an_f[:], in0=an_d[:], in1=dc_d[:], op=ALU.min)
            diff = const.tile([128, MQ], F32)
            nc.vector.tensor_sub(diff[:], ap_d[:], an_f[:])
            lvec = const.tile([128, MQ], F32)
            nc.scalar.activation(out=lvec[:], in_=diff[:], func=ACTF.Relu, bias=marg[:])

            lcol = const.tile([128, 1], F32)
            nc.vector.tensor_reduce(out=lcol[:], in_=lvec[:], axis=AX.X, op=ALU.add)
            lsum = const.tile([128, 1], F32)
            import concourse.bass_isa as bass_isa
            nc.gpsimd.partition_all_reduce(lsum[:], lcol[:], 128, bass_isa.ReduceOp.add)
            tot = const.tile([1, 1], F32)
            nc.vector.tensor_scalar_mul(tot[:], lsum[0:1, :], 1.0 / N)

            if dbg_on:
                dbgt = const.tile([128, 64], F32)
                nc.vector.memset(dbgt[:], 0.0)
                nc.vector.tensor_copy(dbgt[:, 0:NXT], sq_cols[:])
                nc.vector.tensor_copy(dbgt[:, 32:36], apmax[:])
                nc.vector.tensor_copy(dbgt[:, 36:40], anmin[:])
                nc.vector.tensor_copy(dbgt[:, 40:44], wmin[:])
                nc.vector.tensor_copy(dbgt[:, 44:48], sq_q[:])
                nc.vector.tensor_copy(dbgt[:, 48:49], lsum[:])
                nc.vector.tensor_copy(dbgt[:, 49:53], ap_d[:])
                nc.vector.tensor_copy(dbgt[:, 53:57], an_f[:])
                nc.sync.dma_start(out=dbg_h[:, :], in_=dbgt[:])
            if stage_lim >= 3:
                nc.sync.dma_start(out=cc_in[:], in_=tot[:])
                nc.gpsimd.collective_compute(
                    "AllReduce", ALU.add,
                    replica_groups=[list(range(N_CORES))],
                    ins=[cc_in[:]], outs=[cc_out[:]],
                )
                nc.sync.dma_start(out=loss_h[:], in_=cc_out[:])
            else:
                nc.sync.dma_start(out=loss_h[:], in_=tot[:])

    nc.finalize()
    return nc


def _get_nc():
    global _nc_cache
    if _nc_cache is None:
        _nc_cache = _build()
    return _nc_cache


def _in_maps(inputs, targets, center):
    x = np.ascontiguousarray(np.asarray(inputs, dtype=np.float32))
    t = np.ascontiguousarray(np.asarray(targets).astype(np.float32))
    c = np.ascontiguousarray(np.asarray(center, dtype=np.float32))
    assert x.shape == (N, D) and t.shape == (N,) and c.shape == (P, D)
    maps = []
    for core in range(N_CORES):
        s = slice(core * NQ, (core + 1) * NQ)
        maps.append({
            "x": x,
            "xq": np.ascontiguousarray(x[s]),
            "tgt": t,
            "tq": np.ascontiguousarray(t[s]),
            "center": c,
        })
    return maps


def run(inputs, targets, center, trace=False):
    nc = _get_nc()
    res = run_bass_kernel_spmd(
        nc, _in_maps(inputs, targets, center), list(range(N_CORES)), trace=trace
    )
    loss = np.float32(res.results[0]["loss"][0, 0])
    return np.asarray(loss), res


def kernel(inputs, targets, center):
    out, _ = run(inputs, targets, center)
    return out
